# revision 1
# baseline (speedup 1.0000x reference)
"""Trainium2 Bass kernel for nn_CausalPropagationAdjacency.

Shapes (hardcoded): B=4, T=12, N=512, D=128, L=4, H=64.
Pipeline: lag encoders (Linear D->H, ReLU, Linear H->D, mean over L lags),
pairwise scorer sigmoid(relu(src_i+tgt_j+bs1)@Ws2+bs2), threshold 0.1, zero
diagonal, enhanced = A + 0.5 A^2 + 0.25 A^3, normalize by per-batch max.

Sharding: 8 cores = 4 batch-pairs. Core c: batch b=c//2, scores source rows
[half*256, half*256+256) (half=c%2). Adjacency slabs are AllGather'd within
the pair in TWO chunks (the first hides under scoring; a dummy warmup
AllGather at kernel start absorbs the first-collective setup cost). Each core
then computes the full enhanced matrix (hops are cheap) so no second
collective is needed for the global max. Host takes core 2b's output.

SPMD: one program for all cores; per-core behavior differs only through input
data (xlagT = batch lag slices, xsrcT = this core's half), both pre-transposed
to (D-partition, node-free) bf16 by the host.

Pairwise stage: per source i one fused DVE tensor_scalar (add + max0, bf16
out) or ACT Relu-with-bias produces relu(src_i+tgt+bs1) as a (128,512) bf16
tile; a matmul against a 64-wide sliding window of the packed weight buffer
(w2 embedded in one column) accumulates row i%64 of a (64,512) score block in
PSUM — the D-reduction runs at full PE streaming rate. Four score groups give
early sigmoid/threshold completion for the chunked collectives.

Precision: scoring path in bf16 (error ~4e-6 through the sigmoid); the
adjacency crosses the collective as a bf16 RESIDUAL (adj-0.5: values cluster
at 0.5 and exact zeros stay exact, so bf16 keeps ~fp32-level absolute
precision); hops (A^2, A^3) in fp32. End-to-end rel err ~3.9e-5.
"""

import sys
import types
import numpy as np
import ml_dtypes

import concourse.bacc as bacc
import concourse.bass as bass
import concourse.bass_isa as bass_isa
import concourse.mybir as mybir
import concourse.tile as tile
from concourse.bass_utils import run_bass_kernel_spmd

B, T, N, D = 4, 12, 512, 128
L, H = 4, 64
THRESH = 0.1
NCORES = 8
NHALF = N // 2
NT = N // 128
F32 = mybir.dt.float32
BF16 = mybir.dt.bfloat16
AF = mybir.ActivationFunctionType
ALU = mybir.AluOpType

# pairwise engine assignment per i%16 (DVE ~355ns/tile, ACT ~600ns/tile;
# GPSIMD is useless here: 7.6us/tile AND it stalls DVE via the shared port)
ACT_POS = {1, 4, 7, 10, 13}
GP_POS = set()
GP_CUTOFF = 0


def _build_nc():
    nc = bacc.Bacc("TRN2", target_bir_lowering=False, debug=False,
                   num_devices=NCORES)
    xlagT = nc.dram_tensor("xlagT", [L, D, N], BF16, kind="ExternalInput")
    xsrcT = nc.dram_tensor("xsrcT", [L, D, NHALF], BF16, kind="ExternalInput")
    # packed bf16 weights: [w1r(L*H=256) | ws1s(128) | ws1t(128) | zwin(255)
    #   | identity(128) | 0.5*identity(128) | bitcast f32 [bmean|bs1|bs2] (6)]
    wpk = nc.dram_tensor("wpk", [128, 2054], BF16, kind="ExternalInput")
    # w2r (64, L*D) bf16 + b1 (64, L) f32 bitcast to 2*L bf16 cols
    w2r = nc.dram_tensor("w2r", [H, L * D + 2 * L], BF16,
                         kind="ExternalInput")
    outfull = nc.dram_tensor("outfull", [N, N], F32, kind="ExternalOutput")

    with tile.TileContext(nc) as tc:
        _emit(nc, tc, xlagT, xsrcT, wpk, w2r, outfull)
    nc.compile()
    return nc


def _emit(nc, tc, xlagT, xsrcT, wpk, w2r, outfull):
    from contextlib import ExitStack
    ctx = ExitStack()
    with ctx:
        consts = ctx.enter_context(tc.tile_pool(name="consts", bufs=1))
        sb = ctx.enter_context(tc.tile_pool(name="sb", bufs=1))
        relup = ctx.enter_context(tc.tile_pool(name="relu", bufs=10))
        workp = ctx.enter_context(tc.tile_pool(name="work", bufs=4))
        psA = ctx.enter_context(tc.tile_pool(name="psA", bufs=2, space="PSUM"))
        psB = ctx.enter_context(tc.tile_pool(name="psB", bufs=2, space="PSUM"))
        psE = ctx.enter_context(tc.tile_pool(name="psE", bufs=4, space="PSUM"))
        dram = ctx.enter_context(tc.tile_pool(name="dram", bufs=1,
                                              space="DRAM"))

        # ---- input DMAs (few, big; xfull rides the gpsimd queue) ----
        xsrc = consts.tile([D, L, NHALF], BF16, tag="xs")
        nc.sync.dma_start(xsrc[:], xsrcT.ap().rearrange("l d n -> d l n"))
        wpks = consts.tile([128, 2054], BF16, tag="wpk")
        nc.sync.dma_start(wpks[:], wpk[:])
        w2pk = consts.tile([H, L * D + 2 * L], BF16, tag="w2")
        nc.sync.dma_start(w2pk[:], w2r[:])
        xfull = consts.tile([D, L, N], BF16, tag="xf")
        nc.sync.dma_start(xfull[:], xlagT.ap().rearrange("l d n -> d l n"))
        w2sb = w2pk[:, 0:L * D].rearrange("h (l d) -> h l d", l=L)
        b1sb = w2pk[:, L * D:L * D + 2 * L].bitcast(F32)
        w1sb = wpks[:, 0:256].rearrange("d (l h) -> d l h", l=L)
        ws1s_sb = wpks[:, 256:384]
        ws1t_sb = wpks[:, 384:512]
        zw = wpks[:, 512:767]
        idbf = wpks[:, 767:895]
        idhbf = wpks[:, 895:1023]
        fpks = wpks[:, 1024:1030].bitcast(F32)
        idf32 = wpks[:, 1030:1286].bitcast(F32)
        idh32 = wpks[:, 1286:1542].bitcast(F32)
        halfc = wpks[:, 1542:1546].bitcast(F32)  # [:,0]=+0.5, [:,1]=-0.5
        bmean_sb = fpks[:, 0:1]
        bs1_sb = fpks[:, 1:2]
        bs2_sb = fpks[:, 2:3]

        # ---- dummy warmup AllGather: absorbs first-collective setup ----
        warm_in = dram.tile([1, 2], BF16, tag="warmi", name="warm_in")
        warm_out = dram.tile([2, 2], BF16, tag="warmo", name="warm_out")
        nc.gpsimd.dma_start(warm_in[:], wpk[0:1, 0:2])
        nc.gpsimd.collective_compute(
            "AllGather", ALU.bypass,
            replica_groups=[[0, 1], [2, 3], [4, 5], [6, 7]],
            ins=[warm_in.opt()],
            outs=[warm_out.opt()],
        )

        # ---- encoders: (D-part, node) bf16 in, agg out ----
        def encoder(xt, n_nodes, tag):
            encT = psB.tile([D, n_nodes], F32, tag="acc")
            for l in range(L):
                hT = psA.tile([H, n_nodes], F32, tag="t")
                nc.tensor.matmul(hT[:], w1sb[:, l, :], xt[:, l, :],
                                 start=True, stop=True)
                hsb = workp.tile([H, n_nodes], BF16, tag=f"h{tag}")
                nc.scalar.activation(hsb[:], hT[:], AF.Relu,
                                     bias=b1sb[:, l:l + 1], scale=1.0)
                nc.tensor.matmul(encT[:], w2sb[:, l, :], hsb[:],
                                 start=(l == 0), stop=(l == L - 1))
            agg_bf = sb.tile([D, n_nodes], BF16, tag=f"agg{tag}")
            nc.scalar.activation(agg_bf[:], encT[:], AF.Identity,
                                 bias=bmean_sb, scale=1.0 / L)
            return agg_bf

        agg_s = encoder(xsrc, NHALF, "s")
        agg_f = encoder(xfull, N, "f")

        # ---- projections ----
        src_ps = psA.tile([D, NHALF], F32, tag="t")
        nc.tensor.matmul(src_ps[:], ws1s_sb, agg_s[:], start=True,
                         stop=True)
        srcT = sb.tile([D, NHALF], F32, tag="srcf")
        nc.scalar.activation(srcT[:], src_ps[:], AF.Identity,
                             bias=bs1_sb, scale=1.0)
        tgt_ps = psA.tile([D, N], F32, tag="t")
        nc.tensor.matmul(tgt_ps[:], ws1t_sb, agg_f[:], start=True,
                         stop=True)
        tgtT_bf = sb.tile([D, N], BF16, tag="tgtbf")
        nc.vector.tensor_copy(tgtT_bf[:], tgt_ps[:])

        # ---- SBUF homes for gathered adjacency and its transpose ----
        A = [sb.tile([128, N], F32, tag=f"A{kt}", name=f"A{kt}") for kt in range(NT)]
        AT = [sb.tile([128, N], F32, tag=f"AT{kt}", name=f"AT{kt}") for kt in range(NT)]
        a2ps = {}
        a2sb = [sb.tile([128, N], F32, tag=f"a2{it}", name=f"a2sb{it}") for it in range(NT)]
        bounce = [dram.tile([128, N], BF16, tag=f"bnc{c}", name=f"bnc{c}")
                  for c in range(2)]
        full = [dram.tile([256, N], BF16, tag=f"full{c}", name=f"full{c}")
                for c in range(2)]

        # gathered chunk c holds global k-tiles {c, c+2}
        def load_chunk(c):
            nc.gpsimd.collective_compute(
                "AllGather", ALU.bypass,
                replica_groups=[[0, 1], [2, 3], [4, 5], [6, 7]],
                ins=[bounce[c].opt()],
                outs=[full[c].opt()],
            )
            for piece, kt in enumerate((c, c + 2)):
                rsb = workp.tile([128, N], BF16, tag="rsb",
                                 name=f"rsb{c}_{piece}")
                nc.sync.dma_start(
                    rsb[:], full[c][piece * 128:(piece + 1) * 128, :])
                nc.scalar.activation(A[kt][:], rsb[:], AF.Identity,
                                     bias=halfc[:, 0:1], scale=1.0)
                nc.gpsimd.affine_select(
                    A[kt][:], A[kt][:], pattern=[[1, N]],
                    compare_op=ALU.not_equal, fill=0.0,
                    base=-(128 * kt), channel_multiplier=-1)

        def transpose_of(it, kt, use_act):
            """AT[kt][:, it-block] = (A[it][:, kt-block])^T."""
            pool = psA if (it + kt) % 2 == 0 else psB
            tp = pool.tile([128, 128], F32, tag="t" if pool is psA
                           else "acc", name=f"tp{it}_{kt}")
            nc.tensor.transpose(tp[:], A[it][:, kt * 128:(kt + 1) * 128],
                                idf32)
            dst = AT[kt][:, it * 128:(it + 1) * 128]
            if use_act:
                nc.scalar.copy(dst, tp[:])
            else:
                nc.vector.tensor_copy(dst, tp[:])

        def a2_step(it, kt, start, stop):
            if it not in a2ps:
                a2ps[it] = psE.tile([128, N], F32, tag="E",
                                    name=f"a2ps{it}")
            nc.tensor.matmul(a2ps[it][:], AT[kt][:, it * 128:(it + 1) * 128],
                             A[kt][:], start=start, stop=stop)

        # early tail work, sprinkled into the 2nd scoring block:
        # transposes sourced from A[0]/A[2], then a2 partial K-steps {0,2}
        early = []
        for it in (0, 2):
            for kt in range(NT):
                early.append(
                    lambda it=it, kt=kt: transpose_of(it, kt, (it + kt) % 2))
        for it in (0, 2):
            for kt in (0, 2):
                early.append(
                    lambda it=it, kt=kt: a2_step(it, kt, kt == 0, False))

        # ---- pairwise scoring: 4 groups of 64 source rows (M=64) ----
        # w2 sits at wpk column 639; window [639-p : 703-p] puts it in
        # column p of a 64-wide lhsT -> psum row p of the (64,512) group
        for g in range(4):
            score_ps = psB.tile([64, N], F32, tag="acc", name=f"scps{g}")
            for p in range(64):
                i = g * 64 + p
                rt = relup.tile([D, N], BF16, tag="rt")
                act_pos = ACT_POS | ({9} if i < 128 else set())
                if i % 16 in act_pos:
                    nc.scalar.activation(rt[:], tgtT_bf[:], AF.Relu,
                                         bias=srcT[:, i:i + 1], scale=1.0)
                else:
                    nc.vector.tensor_scalar(rt[:], tgtT_bf[:],
                                            srcT[:, i:i + 1], 0.0,
                                            ALU.add, ALU.max)
                nc.tensor.matmul(score_ps[:], wpks[:, 639 - p:703 - p],
                                 rt[:], start=(p == 0), stop=(p == 63))
            score_sb = workp.tile([64, N], F32, tag="score",
                                  name=f"scsb{g}")
            nc.scalar.activation(score_sb[:], score_ps[:], AF.Sigmoid,
                                 bias=bs2_sb[0:64, :], scale=1.0)
            adjs = workp.tile([64, N], F32, tag="adjs", name=f"adj{g}")
            nc.vector.scalar_tensor_tensor(adjs[:], score_sb[:], THRESH,
                                           score_sb[:], ALU.is_gt, ALU.mult)
            # residual encode: adj values cluster near 0.5 (and exact 0);
            # adj-0.5 in bf16 keeps ~fp32-level absolute precision here
            resid = workp.tile([64, N], BF16, tag="resid", name=f"rs{g}")
            nc.scalar.activation(resid[:], adjs[:], AF.Identity,
                                 bias=halfc[0:64, 1:2], scale=1.0)
            nc.sync.dma_start(bounce[g // 2][(g % 2) * 64:(g % 2) * 64 + 64, :],
                              resid[:])
            if g % 2 == 1:
                load_chunk(g // 2)
        # early-tail ops (A[0]/A[2] transposes + partial a2): run in the
        # collective-#2 window; gated only on chunk 0, which is long done
        while early:
            early.pop(0)()
        # ---- late tail, emitted in dependency-readiness order ----
        # a2[0], a2[2] late K-steps only need A[1]/A[3] as rhs (their AT
        # slices came from A[0]/A[2], transposed early) -> finish + evac first
        for it in (0, 2):
            a2_step(it, 1, False, False)
            a2_step(it, 3, False, True)
            nc.vector.tensor_copy(a2sb[it][:], a2ps[it][:])
        # transposes sourced from A[1]/A[3], then a2[1], a2[3]
        for it in (1, 3):
            for kt in range(NT):
                transpose_of(it, kt, (it + kt) % 2 == 0)
        for it in (1, 3):
            for kt in range(NT):
                a2_step(it, kt, kt == 0, kt == 3)
            nc.vector.tensor_copy(a2sb[it][:], a2ps[it][:])

        # ---- E = A@a2 + 0.5*a2 + A, accumulated in PSUM ----
        E = []
        mx4 = sb.tile([128, NT], F32, tag="mx4")
        for it in range(NT):
            e_ps = psE.tile([128, N], F32, tag="E")
            nc.tensor.matmul(e_ps[:], idf32, A[it][:], start=True, stop=False)
            nc.tensor.matmul(e_ps[:], idh32, a2sb[it][:], start=False,
                             stop=False)
            for kt in range(NT):
                nc.tensor.matmul(e_ps[:], AT[kt][:, it * 128:(it + 1) * 128],
                                 a2sb[kt][:], start=False, stop=(kt == 3))
            nc.vector.reduce_max(mx4[:, it:it + 1], e_ps[:],
                                 axis=mybir.AxisListType.X)
            E.append(e_ps)

        # ---- global max + normalize + write out ----
        mxp = sb.tile([128, 1], F32, tag="mxp")
        nc.vector.reduce_max(mxp[:], mx4[:], axis=mybir.AxisListType.X)
        mxall = sb.tile([128, 1], F32, tag="mxall")
        nc.gpsimd.partition_all_reduce(mxall[:], mxp[:], 128,
                                       bass_isa.ReduceOp.max)
        denom = sb.tile([128, 1], F32, tag="denom")
        nc.vector.tensor_scalar(denom[:], mxall[:], 1e-8, None, ALU.add)
        recip = sb.tile([128, 1], F32, tag="recip")
        nc.vector.reciprocal(recip[:], denom[:])
        for it in range(NT):
            ot = workp.tile([128, N], F32, tag="ot")
            if it % 2 == 0:
                nc.vector.tensor_scalar(ot[:], E[it][:], recip[:, 0:1], None,
                                        ALU.mult)
            else:
                nc.scalar.mul(ot[:], E[it][:], recip[:, 0:1])
            nc.sync.dma_start(outfull[it * 128:(it + 1) * 128, :], ot[:])


_NC_CACHE = {}


def _get_nc():
    if "nc" not in _NC_CACHE:
        _NC_CACHE["nc"] = _build_nc()
    return _NC_CACHE["nc"]


def _install_ntff_hook():
    try:
        from antenv.axon_hooks import get_axon_ntff_profile_hook  # noqa: F401
        return
    except ImportError:
        pass
    try:
        import importlib.util
        spec = importlib.util.spec_from_file_location(
            "trn_boot_mod", "/root/.axon_site/trn_agent_boot/trn_boot.py")
        tb = importlib.util.module_from_spec(spec)
        spec.loader.exec_module(tb)
        hook = tb._ntff_profile_via_ctypes("/opt/axon/libaxon_pjrt.so")
        m = types.ModuleType("antenv.axon_hooks")
        m.get_axon_ntff_profile_hook = lambda: hook
        m.set_axon_ntff_profile_hook = lambda h: None
        sys.modules["antenv.axon_hooks"] = m
    except Exception:
        pass


def _bf(a):
    return np.ascontiguousarray(a).astype(ml_dtypes.bfloat16)


def _prep_in_maps(x, W1, b1, W2, b2, Ws1, bs1, Ws2, bs2):
    x = np.asarray(x, np.float32)
    W1 = np.asarray(W1, np.float32)
    b1 = np.asarray(b1, np.float32)
    W2 = np.asarray(W2, np.float32)
    b2 = np.asarray(b2, np.float32)
    Ws1 = np.asarray(Ws1, np.float32)
    bs1 = np.asarray(bs1, np.float32)
    Ws2 = np.asarray(Ws2, np.float32)
    bs2 = np.asarray(bs2, np.float32)

    Tdim = x.shape[1]
    lag_idx = [max(0, Tdim - 1 - l) for l in range(L)]
    xl = x[:, lag_idx]                            # (B, L, N, D)
    xlT = np.swapaxes(xl, 2, 3)                   # (B, L, D, N)

    zwin = np.zeros((128, 255), np.float32)
    zwin[:, 127] = Ws2[:, 0]
    fpk = np.stack([b2.mean(axis=0), bs1,
                    np.full(128, bs2[0], np.float32)], axis=1)
    fpk_bf = np.ascontiguousarray(fpk.astype(np.float32)).view(
        ml_dtypes.bfloat16)                               # (128, 6)
    wpk = np.concatenate([
        _bf(np.transpose(W1, (1, 0, 2)).reshape(D, L * H)),
        _bf(Ws1[:D]),
        _bf(Ws1[D:]),
        _bf(zwin),
        _bf(np.eye(128, dtype=np.float32)),
        _bf(0.5 * np.eye(128, dtype=np.float32)),
        np.zeros((128, 1), ml_dtypes.bfloat16),           # pad to even col
        fpk_bf,
        np.eye(128, dtype=np.float32).view(ml_dtypes.bfloat16),
        (0.5 * np.eye(128, dtype=np.float32)).view(ml_dtypes.bfloat16),
        np.concatenate([np.full((128, 1), 0.5, np.float32),
                        np.full((128, 1), -0.5, np.float32)],
                       axis=1).view(ml_dtypes.bfloat16),
        np.zeros((128, 2054 - 1546), ml_dtypes.bfloat16),
    ], axis=1)                                            # (128, 2054)
    b1_bf = np.ascontiguousarray(b1.T.astype(np.float32)).view(
        ml_dtypes.bfloat16)                               # (64, 2L)
    w2pk = np.concatenate(
        [_bf(np.transpose(W2, (1, 0, 2)).reshape(H, L * D)), b1_bf], axis=1)

    common = {
        "wpk": np.ascontiguousarray(wpk),
        "w2r": np.ascontiguousarray(w2pk),
    }
    in_maps = []
    for c in range(NCORES):
        b, half = c // 2, c % 2
        m = dict(common)
        m["xlagT"] = _bf(xlT[b])
        m["xsrcT"] = _bf(xlT[b][:, :, half * NHALF:(half + 1) * NHALF])
        in_maps.append(m)
    return in_maps


def _run(inputs, trace=False):
    nc = _get_nc()
    in_maps = _prep_in_maps(**inputs)
    if trace:
        _install_ntff_hook()
    res = run_bass_kernel_spmd(nc, in_maps, core_ids=list(range(NCORES)),
                               trace=trace)
    out = np.stack([res.results[2 * b]["outfull"] for b in range(B)], axis=0)
    return out, res


def kernel(**inputs):
    out, _ = _run(inputs, trace=False)
    return out

